# revision 42
# baseline (speedup 1.0000x reference)
"""Trainium2 Bass kernel for a pre-norm transformer block (dense_transformer).

Computation (per reference):
    x = x + Attn(LN1(x));  x = x + MLP(LN2(x))
with causal multi-head attention (H=16 heads, D=64) and a 4E ReLU MLP.

Sharding: DP-2 on batch x TP-4 on heads.  Core c = b*4 + r computes
LN1(x_b) over all T tokens, Q/K/V + causal attention for heads 4r..4r+3
only (so K/V projections are not recomputed 4x and score tiles above the
causal diagonal are skipped entirely), then the partial out-projection
for all tokens.  A single bf16 ReduceScatter over each 4-core group sums
the head-group partials and hands core r its 512-token slice, on which it
runs residual + LN2 + the full 4E MLP.

Layouts are feature-major throughout (E on partitions, tokens on the free
axis).  Softmax row sums come free from a ones-column appended to V
(M=65 attnV matmuls).  All weights are host-packed into lhsT layout so
every DMA is contiguous.  Matmuls run in bf16 with fp32 PSUM.
"""

from dataclasses import dataclass

import numpy as np
import ml_dtypes

import concourse.bass as bass  # noqa: F401
import concourse.mybir as mybir
import concourse.tile as tile
from concourse import bacc
from concourse import bass_utils

F32 = mybir.dt.float32
BF16 = mybir.dt.bfloat16
AF = mybir.ActivationFunctionType
OP = mybir.AluOpType
NPBF16 = ml_dtypes.bfloat16

P = 128


@dataclass(frozen=True)
class Cfg:
    B: int = 2
    T: int = 2048
    E: int = 1024
    H: int = 16
    D: int = 64
    NC: int = 8
    eps: float = 1e-5

    @property
    def CPB(self):  # cores per batch (TP group size)
        return self.NC // self.B

    @property
    def Tq(self):  # tokens owned per core (MLP stage)
        return self.T // self.CPB

    @property
    def KE(self):  # E / 128
        return self.E // P

    @property
    def TK(self):  # T / 128 context tiles
        return self.T // P

    @property
    def HPC(self):  # heads per core
        return self.H // self.CPB

    @property
    def JC(self):  # 128-row head-pair blocks per core
        return self.HPC // 2

    @property
    def F(self):
        return 4 * self.E

    @property
    def KF(self):
        return self.F // P

    @property
    def NQC(self):  # query chunks of Tq
        return self.T // self.Tq

    def check(self):
        assert self.D == 64 and self.E == self.H * self.D
        assert self.Tq == 512 and self.HPC == 4 and self.JC == 2
        assert self.T % P == 0 and self.E % P == 0 and self.F % P == 0


CFG = Cfg()


class Pools:
    """Tile pools with explicit open/close (LIFO per side, per space)."""

    def __init__(self, tc, prefix=""):
        self.tc = tc
        self.prefix = prefix
        self.live = {}

    def open(self, key, bufs, space=None, side=None):
        kw = dict(name=self.prefix + key, bufs=bufs)
        if space:
            kw["space"] = space
        if side:
            kw["side"] = side
        cm = self.tc.tile_pool(**kw)
        pool = cm.__enter__()
        self.live[key] = cm
        return pool

    def close(self, *keys):
        for key in keys:
            self.live.pop(key).__exit__(None, None, None)

    def close_all(self):
        for key in reversed(list(self.live)):
            self.close(key)


def _emit(tc, c: Cfg, d):
    nc = tc.nc
    E, T, Tq = c.E, c.T, c.Tq
    KE, TK, KF, JC, NQC, HPC = c.KE, c.TK, c.KF, c.JC, c.NQC, c.HPC
    DV = 65  # V cols per head incl. ones column
    SCL = 1.0 / float(np.sqrt(c.D))

    pp = Pools(tc)

    # ---------------- constants (whole-kernel lifetime) --------------------
    const = pp.open("const", 1)
    ones_bf = const.tile([P, 1], BF16, name="ones_bf")
    nc.vector.memset(ones_bf[:], 1.0)
    sel64 = const.tile([4, 4 * 64], BF16, name="sel64")
    nc.sync.dma_start(sel64[:], d["sel64"])
    gbt = {}
    for nm, cols in [
        ("ln1g", KE), ("ln1b", KE), ("ln2g", KE), ("ln2b", KE),
        ("boc", KE), ("mb1", KF), ("mb2", KE),
    ]:
        gbt[nm] = const.tile([P, cols], F32, name=nm + "_t")
        nc.sync.dma_start(gbt[nm][:], d[nm])

    ident = const.tile([P, P], BF16, name="ident")
    nc.sync.dma_start(ident[:], d["ident"])
    wrm = const.tile([P, Tq], BF16, name="wrm")
    nc.vector.memset(wrm[:], 0.0)

    p_band = pp.open("bandp", 1)
    mask01 = p_band.tile([P, 4 * Tq], BF16, name="mask01")
    nc.sync.dma_start(mask01[:], d["mask01"])

    # ---------------- DRAM bounce buffers for the ReduceScatters -----------
    # one RS per query chunk; rank r of each 4-core group receives token
    # stripe r (128 tokens) of that chunk, so the collectives pipeline under
    # the remaining attention compute.
    p_dram = pp.open("dram", 1, "DRAM")
    bin_q = [p_dram.tile([4 * E, P], BF16, name=f"bin{qc}") for qc in range(NQC)]
    bout_q = [p_dram.tile([E, P], BF16, name=f"bout{qc}") for qc in range(NQC)]

    # ---------------- PE warmup (pstate ramp) ------------------------------
    ps_wm = pp.open("warm_ps", 1, "PSUM")
    wmp = ps_wm.tile([1, Tq], F32, name="wmp")
    for _w in range(24):
        nc.tensor.matmul(wmp[:], ones_bf[:], wrm[:], start=True, stop=True)
    pp.close("warm_ps")

    # ======================================================================
    # Phase 0: load x^T (bf16) + LayerNorm1 over all T -> xn (bf16)
    # ======================================================================
    p_kq = pp.open("kqp", 1)
    p_vs = pp.open("vsp", 1)
    qt = [p_kq.tile([P, T], BF16, name=f"qt{j}") for j in range(JC)]
    kt = [p_kq.tile([P, T], BF16, name=f"kt{j}") for j in range(JC)]
    vsb = p_vs.tile([P, TK, HPC, DV], BF16, name="vsb")
    nc.vector.memset(vsb[:, :, :, DV - 1 : DV], 1.0)

    p_xn = pp.open("xnp", 1)
    p_w3 = pp.open("w3p", 1)
    wqt = p_w3.tile([P, KE, 2 * P], BF16, name="wqt")
    wkt = p_w3.tile([P, KE, 2 * P], BF16, name="wkt")
    wvt = p_w3.tile([P, KE, 2 * P], BF16, name="wvt")
    nc.sync.dma_start(wkt[:], d["wk"])
    p_xt = pp.open("xtp", 1)
    xt = [p_xt.tile([P, T], BF16, name=f"xt{e}") for e in range(KE)]
    xn = [p_xn.tile([P, T], BF16, name=f"xn{e}") for e in range(KE)]
    for e in range(KE):
        nc.sync.dma_start(xt[e][:], d["xt"][e * P : (e + 1) * P, :])
    nc.sync.dma_start(wqt[:], d["wq"])
    nc.sync.dma_start(wvt[:], d["wv"])

    ps_qkv = pp.open("qkv_ps", 2, "PSUM")
    p_tmp = pp.open("ln_tmp", 4)
    p_rows = pp.open("ln_rows", 1)
    p_bcs = pp.open("ln_bcs", 2)
    ps_st = pp.open("ln_st", 1, "PSUM")

    def kq_proj(ci):
        cs = slice(ci * Tq, (ci + 1) * Tq)
        for j in range(JC):
            for w_t, dst in ((wkt, kt), (wqt, qt)):
                ps = ps_qkv.tile([P, Tq], F32, name="kq_ps")
                for e in range(KE):
                    nc.tensor.matmul(
                        ps[:], w_t[:, e, j * P : (j + 1) * P], xn[e][:, cs],
                        start=(e == 0), stop=(e == KE - 1),
                    )
                nc.scalar.copy(dst[j][:, cs], ps[:])

    for ci in range(NQC):
        cs = slice(ci * Tq, (ci + 1) * Tq)
        s1 = ps_st.tile([1, Tq], F32, name="s1")
        s2 = ps_st.tile([1, Tq], F32, name="s2")
        for e in range(KE):
            x2 = p_tmp.tile([P, Tq], BF16, name="x2bf")
            nc.scalar.square(x2[:], xt[e][:, cs])
            nc.tensor.matmul(s1[:], ones_bf[:], xt[e][:, cs], start=(e == 0), stop=(e == KE - 1))
            nc.tensor.matmul(s2[:], ones_bf[:], x2[:], start=(e == 0), stop=(e == KE - 1))
        mu = p_rows.tile([1, Tq], F32, name="mu")
        nc.vector.tensor_scalar_mul(mu[:], s1[:], 1.0 / E)
        ve = p_rows.tile([1, Tq], F32, name="ve")
        nc.vector.tensor_scalar(ve[:], s2[:], 1.0 / E, c.eps, OP.mult, OP.add)
        mu2 = p_rows.tile([1, Tq], F32, name="mu2")
        nc.vector.tensor_tensor(mu2[:], mu[:], mu[:], OP.mult)
        vee = p_rows.tile([1, Tq], F32, name="vee")
        nc.vector.tensor_tensor(vee[:], ve[:], mu2[:], OP.subtract)
        lv = p_rows.tile([1, Tq], F32, name="lv")
        nc.scalar.activation(lv[:], vee[:], AF.Ln)
        rstd = p_rows.tile([1, Tq], F32, name="rstd")
        nc.scalar.activation(rstd[:], lv[:], AF.Exp, scale=-0.5)

        mub = p_bcs.tile([P, Tq], F32, name="mub")
        nc.gpsimd.partition_broadcast(mub[:], mu[:])
        rsb = p_bcs.tile([P, Tq], F32, name="rsb")
        nc.gpsimd.partition_broadcast(rsb[:], rstd[:])

        for e in range(KE):
            # alternate DVE / gpsimd so the normalize stream isn't one-engine
            eng = nc.vector if e % 2 == 0 else nc.gpsimd
            t1 = p_tmp.tile([P, Tq], F32, name=f"t1_{e % 2}")
            eng.tensor_tensor(t1[:], xt[e][:, cs], mub[:], OP.subtract)
            t2 = p_tmp.tile([P, Tq], F32, name=f"t2_{e % 2}")
            eng.tensor_tensor(t2[:], t1[:], rsb[:], OP.mult)
            eng.tensor_scalar(
                xn[e][:, cs], t2[:],
                gbt["ln1g"][:, e : e + 1], gbt["ln1b"][:, e : e + 1],
                OP.mult, OP.add,
            )
        if ci >= 1:
            kq_proj(ci - 1)
    kq_proj(NQC - 1)
    pp.close("ln_st", "ln_bcs", "ln_rows", "ln_tmp", "xtp")

    # ======================================================================
    # Phase 1: V projection (K/Q were interleaved with LN1 above)
    # ======================================================================
    for t in range(TK):
        ps = ps_qkv.tile([P, 2 * P], F32, name="v_ps")
        for e in range(KE):
            nc.tensor.matmul(
                ps[:], xn[e][:, t * P : (t + 1) * P], wvt[:, e, :],
                start=(e == 0), stop=(e == KE - 1),
            )
        nc.scalar.copy(
            vsb[:, t, :, 0:64],
            ps[:].rearrange("p (h v) -> p h v", h=HPC),
        )
    pp.close("qkv_ps", "w3p", "xnp")

    # prefetch the out-proj + MLP weights + residual slice while attention runs
    p_wo = pp.open("wop", 1, side="right")
    wot = p_wo.tile([P, JC, E], BF16, name="wot")
    nc.scalar.dma_start(wot[:], d["wo"])
    p_w12 = pp.open("w12p", 1, side="right")
    w1t = p_w12.tile([P, KE, c.F], BF16, name="w1t")
    w2t = p_w12.tile([P, KF, E], BF16, name="w2t")
    # w1/w2 are loaded in 1MB chunks interleaved into the attention stream
    # (see finalize) so they never monopolize the DMA engines or queues
    wload = []
    for e in range(KE):
        wload.append((w1t[:, e, :], d["w1"][:, e, :]))
    for fb in range(KE):
        wload.append((
            w2t[:, 4 * fb : 4 * (fb + 1), :], d["w2"][:, 4 * fb : 4 * (fb + 1), :]
        ))
    # ======================================================================
    # Phase 2: causal attention for 4 heads (2 pairs), all query chunks
    # ======================================================================
    p_ao = pp.open("aop", 1)
    p_pr = pp.open("probs", 4)
    p_rst = pp.open("rsst", 1)
    p_st2 = pp.open("rstage", 2)
    ps_av = pp.open("av_ps", 1, "PSUM")
    ps_ss = pp.open("ss_ps", 2, "PSUM")

    all_units = [
        (qc, t, p) for qc in range(NQC) for t in range(4 * qc + 4) for p in range(JC)
    ]
    LOOK = 2  # score-lookahead units (PSUM ring self-regulates via WAR)
    avp = {}

    def emit_ss(i):
        qc, t, p = all_units[i]
        jb = t - 4 * qc  # >= 0 on the causal diagonal band
        qs = slice(qc * Tq, (qc + 1) * Tq)
        ssu = ps_ss.tile([P, 2 * Tq], F32, name="ss")
        for s in (0, 1):
            nc.tensor.matmul(
                ssu[:, s * Tq : (s + 1) * Tq],
                kt[p][s * 64 : (s + 1) * 64, t * P : (t + 1) * P],
                qt[p][s * 64 : (s + 1) * 64, qs],
                start=True, stop=(jb < 0),
                tile_position=(s * 64, 0),
                skip_group_check=True,
            )
        if jb >= 0:
            # add -1e9 above the diagonal straight into the score PSUM
            for s in (0, 1):
                nc.tensor.matmul(
                    ssu[:, s * Tq : (s + 1) * Tq], ident[:],
                    mask01[:, jb * Tq : (jb + 1) * Tq],
                    start=False, stop=True,
                    skip_group_check=True,
                )
        return ssu

    aop_all = {}

    def finalize_part1(qc):
        """Copy the accumulators out of PSUM right away to unblock the next
        chunk's avp matmuls; the rest of the finalize is emitted a few units
        into the next chunk so its latency hides under the running stream."""
        avs = p_rst.tile([P, HPC * Tq], F32, name="avs")
        for h in range(HPC):
            nc.vector.tensor_copy(avs[0:DV, h * Tq : (h + 1) * Tq], avp[h][:])
        return avs

    def finalize_part2a(qc, avs):
        rs4 = p_st2.tile([4, Tq], F32, name="rs4", bufs=1)
        nc.sync.dma_start(rs4[:], avs[64:65, :])
        lrs = p_st2.tile([4, Tq], F32, name="lrs", bufs=1)
        nc.scalar.activation(lrs[:], rs4[:], AF.Ln)
        irs = p_st2.tile([4, Tq], BF16, name="irs", bufs=1)
        nc.scalar.activation(irs[:], lrs[:], AF.Exp, scale=-1.0)

        aop = [p_ao.tile([P, Tq], BF16, name=f"aop{qc}_{p}") for p in range(JC)]
        for p in range(JC):
            nb = ps_ss.tile([P, 2 * Tq], F32, name="ss")
            for s in (0, 1):
                nc.tensor.matmul(
                    nb[0:64, s * Tq : (s + 1) * Tq],
                    sel64[:, (2 * p + s) * 64 : (2 * p + s + 1) * 64],
                    irs[:],
                    start=True, stop=True,
                    skip_group_check=True,
                )
            nbs = p_st2.tile([64, 2 * Tq], BF16, name="nbs", bufs=1)
            nc.vector.tensor_copy(nbs[:], nb[0:64, :])
            nc.gpsimd.tensor_tensor(
                aop[p][0:64, :], avs[0:64, 2 * p * Tq : (2 * p + 1) * Tq],
                nbs[:, 0:Tq], OP.mult,
            )
            ost = p_st2.tile([64, Tq], BF16, name="ost")
            nc.gpsimd.tensor_tensor(
                ost[:], avs[0:64, (2 * p + 1) * Tq : (2 * p + 2) * Tq],
                nbs[:, Tq : 2 * Tq], OP.mult,
            )
            nc.sync.dma_start(aop[p][64:128, :], ost[:])
        return aop

    def finalize_part2b(qc, aop, last):
        obq = p_st2.tile([P, KE, Tq], BF16, name="obq", bufs=1)
        for e in range(KE):
            po = ps_ss.tile([P, 2 * Tq], F32, name="ss")
            for p in range(JC):
                nc.tensor.matmul(
                    po[:, 0:Tq], wot[:, p, e * P : (e + 1) * P], aop[p][:],
                    start=(p == 0), stop=(p == JC - 1),
                    skip_group_check=True,
                )
            if last and e % 2 == 1:
                nc.scalar.copy(obq[:, e, :], po[:, 0:Tq])
            else:
                nc.vector.tensor_copy(obq[:, e, :], po[:, 0:Tq])
        for j in range(4):
            nc.sync.dma_start(
                bin_q[qc][j * E : (j + 1) * E, :].rearrange("(e p) c -> p e c", p=P),
                obq[:, :, j * P : (j + 1) * P],
            )
        nc.gpsimd.collective_compute(
            "ReduceScatter",
            OP.add,
            replica_groups=[[0, 1, 2, 3], [4, 5, 6, 7]],
            ins=[bin_q[qc].opt()],
            outs=[bout_q[qc].opt()],
        )

        for _ in range(4):
            if wload:
                dst, src = wload.pop(0)
                nc.sync.dma_start(dst, src)

    ss_ring = [emit_ss(i) for i in range(min(LOOK, len(all_units)))]
    pend_a = None  # (qc, avs, waited)
    pend_b = None  # (qc, aop, waited)
    for i, (qc, t, p) in enumerate(all_units):
        ntile = 4 * qc + 4
        if t == 0 and p == 0:
            for h in range(HPC):
                avp[h] = ps_av.tile([DV, Tq], F32, name=f"avp{h}")
        pr = p_pr.tile([P, 2 * Tq], BF16, name="pr")
        nc.scalar.activation(pr[:], ss_ring[0][:], AF.Exp, scale=SCL)
        ss_ring.pop(0)
        if i + LOOK < len(all_units):
            ss_ring.append(emit_ss(i + LOOK))
        for s in (0, 1):
            h = 2 * p + s
            nc.tensor.matmul(
                avp[h][:],
                vsb[:, t, h, :],
                pr[:, s * Tq : (s + 1) * Tq],
                start=(t == 0), stop=(t == ntile - 1),
                skip_group_check=True,
            )
        if pend_a is not None:
            fqc, favs, waited = pend_a
            if waited >= 2:
                pend_b = (fqc, finalize_part2a(fqc, favs), 0)
                pend_a = None
            else:
                pend_a = (fqc, favs, waited + 1)
        elif pend_b is not None:
            fqc, faop, waited = pend_b
            if waited >= 4:
                finalize_part2b(fqc, faop, last=False)
                pend_b = None
            else:
                pend_b = (fqc, faop, waited + 1)
        if t == ntile - 1 and p == JC - 1:
            avs = finalize_part1(qc)
            if i == len(all_units) - 1:
                if pend_b is not None:
                    finalize_part2b(pend_b[0], pend_b[1], last=False)
                    pend_b = None
                finalize_part2b(qc, finalize_part2a(qc, avs), last=True)
            else:
                pend_a = (qc, avs, 0)
    if pend_b is not None:
        finalize_part2b(pend_b[0], pend_b[1], last=True)
        pend_b = None

    pp.close("ss_ps", "av_ps", "rstage", "rsst", "probs", "aop")
    pp.close("vsp", "kqp", "bandp")

    # ======================================================================
    # Phase 3: ReduceScatter partials; residual + bo -> xres; LN2 -> xn2
    # ======================================================================
    p_xo = pp.open("xop", 1, side="right")
    xown = [p_xo.tile([P, Tq], F32, name=f"xown{e}") for e in range(KE)]
    for e in range(KE):
        nc.sync.dma_start(xown[e][:], d["x_own"][e * P : (e + 1) * P, :])

    p_xr = pp.open("xrp", 1)
    p_x2 = pp.open("xn2p", 1)
    p_att = pp.open("attp", 1)
    xres = [p_xr.tile([P, Tq], F32, name=f"xres{e}") for e in range(KE)]
    xn2 = [p_x2.tile([P, Tq], BF16, name=f"xn2{e}") for e in range(KE)]
    att = p_att.tile([P, KE, Tq], BF16, name="att")
    for qc in range(NQC):
        nc.sync.dma_start(
            att[:, :, qc * P : (qc + 1) * P],
            bout_q[qc].rearrange("(e p) c -> p e c", p=P),
        )
    for e in range(KE):
        nc.vector.scalar_tensor_tensor(
            xres[e][:], att[:, e, :], gbt["boc"][:, e : e + 1], xown[e][:],
            OP.add, OP.add,
        )
    pp.close("attp")
    pp.close("xop")

    p_tmp = pp.open("ln2_tmp", 3)
    p_rows = pp.open("ln2_rows", 1)
    p_bc2 = pp.open("ln2_bcs", 1)
    ps_st = pp.open("ln2_st", 1, "PSUM")
    s1 = ps_st.tile([1, Tq], F32, name="s1b")
    s2 = ps_st.tile([1, Tq], F32, name="s2b")
    for e in range(KE):
        xbf = p_tmp.tile([P, Tq], BF16, name="xbf2")
        nc.vector.tensor_copy(xbf[:], xres[e][:])
        x2 = p_tmp.tile([P, Tq], BF16, name="x2bf2")
        nc.scalar.square(x2[:], xres[e][:])
        nc.tensor.matmul(s1[:], ones_bf[:], xbf[:], start=(e == 0), stop=(e == KE - 1))
        nc.tensor.matmul(s2[:], ones_bf[:], x2[:], start=(e == 0), stop=(e == KE - 1))
    mu = p_rows.tile([1, Tq], F32, name="mu_2")
    nc.vector.tensor_scalar_mul(mu[:], s1[:], 1.0 / E)
    ve = p_rows.tile([1, Tq], F32, name="ve_2")
    nc.vector.tensor_scalar(ve[:], s2[:], 1.0 / E, c.eps, OP.mult, OP.add)
    mu2 = p_rows.tile([1, Tq], F32, name="mu2_2")
    nc.vector.tensor_tensor(mu2[:], mu[:], mu[:], OP.mult)
    vee = p_rows.tile([1, Tq], F32, name="vee_2")
    nc.vector.tensor_tensor(vee[:], ve[:], mu2[:], OP.subtract)
    lv = p_rows.tile([1, Tq], F32, name="lv_2")
    nc.scalar.activation(lv[:], vee[:], AF.Ln)
    rstd = p_rows.tile([1, Tq], F32, name="rstd_2")
    nc.scalar.activation(rstd[:], lv[:], AF.Exp, scale=-0.5)
    mub = p_bc2.tile([P, Tq], F32, name="mub2")
    nc.gpsimd.partition_broadcast(mub[:], mu[:])
    rsb = p_bc2.tile([P, Tq], F32, name="rsb2")
    nc.gpsimd.partition_broadcast(rsb[:], rstd[:])
    for e in range(KE):
        eng = nc.vector if e % 2 == 0 else nc.gpsimd
        t1 = p_tmp.tile([P, Tq], F32, name=f"t1b_{e % 2}")
        eng.tensor_tensor(t1[:], xres[e][:], mub[:], OP.subtract)
        t2 = p_tmp.tile([P, Tq], F32, name=f"t2b_{e % 2}")
        eng.tensor_tensor(t2[:], t1[:], rsb[:], OP.mult)
        eng.tensor_scalar(
            xn2[e][:], t2[:],
            gbt["ln2g"][:, e : e + 1], gbt["ln2b"][:, e : e + 1],
            OP.mult, OP.add,
        )
    pp.close("ln2_st", "ln2_bcs", "ln2_rows", "ln2_tmp")

    # ======================================================================
    # Phase 4: MLP (layer 1 streamed with first 6 e-tiles of layer 2)
    # ======================================================================
    EH = min(KE, 6)
    p_h1 = pp.open("h1p", 1, side="right")
    p_out = pp.open("outp", 2)
    ps_h1 = pp.open("h1_ps", 2, "PSUM")
    ps_h2a = pp.open("h2a_ps", 1, "PSUM")

    h1 = [p_h1.tile([P, Tq], BF16, name=f"h1{f}") for f in range(KF)]
    h2a = [ps_h2a.tile([P, Tq], F32, name=f"h2a{e}") for e in range(EH)]
    for f in range(KF):
        ps = ps_h1.tile([P, Tq], F32, name="h1ps")
        for e in range(KE):
            nc.tensor.matmul(
                ps[:], w1t[:, e, f * P : (f + 1) * P], xn2[e][:],
                start=(e == 0), stop=(e == KE - 1),
            )
        nc.scalar.activation(
            h1[f][:], ps[:], AF.Relu, bias=gbt["mb1"][:, f : f + 1], scale=1.0
        )
        for e in range(EH):
            nc.tensor.matmul(
                h2a[e][:], w2t[:, f, e * P : (e + 1) * P], h1[f][:],
                start=(f == 0), stop=(f == KF - 1),
            )
    for e in range(EH):
        of = p_out.tile([P, Tq], F32, name="outf")
        nc.vector.scalar_tensor_tensor(
            of[:], h2a[e][:], gbt["mb2"][:, e : e + 1], xres[e][:], OP.add, OP.add
        )
        nc.sync.dma_start(d["out_t"][e * P : (e + 1) * P, :], of[:])
    pp.close("h2a_ps", "h1_ps")

    if EH < KE:
        ps_h2b = pp.open("h2b_ps", 1, "PSUM")
        h2b = [ps_h2b.tile([P, Tq], F32, name=f"h2b{e}") for e in range(KE - EH)]
        for f in range(KF):
            for i, e in enumerate(range(EH, KE)):
                nc.tensor.matmul(
                    h2b[i][:], w2t[:, f, e * P : (e + 1) * P], h1[f][:],
                    start=(f == 0), stop=(f == KF - 1),
                )
        for i, e in enumerate(range(EH, KE)):
            of = p_out.tile([P, Tq], F32, name="outf")
            nc.vector.scalar_tensor_tensor(
                of[:], h2b[i][:], gbt["mb2"][:, e : e + 1], xres[e][:], OP.add, OP.add
            )
            nc.sync.dma_start(d["out_t"][e * P : (e + 1) * P, :], of[:])

    pp.close_all()


def build_program(c: Cfg = CFG):
    c.check()
    nc = bacc.Bacc(
        "TRN2",
        target_bir_lowering=False,
        debug=False,
        enable_asserts=False,
        num_devices=c.NC,
    )
    d = {}
    d["xt"] = nc.dram_tensor("xt", [c.E, c.T], BF16, kind="ExternalInput").ap()
    d["x_own"] = nc.dram_tensor("x_own", [c.E, c.Tq], F32, kind="ExternalInput").ap()
    d["wq"] = nc.dram_tensor("wq", [P, c.KE, 2 * P], BF16, kind="ExternalInput").ap()
    d["wk"] = nc.dram_tensor("wk", [P, c.KE, 2 * P], BF16, kind="ExternalInput").ap()
    d["wv"] = nc.dram_tensor("wv", [P, c.KE, 2 * P], BF16, kind="ExternalInput").ap()
    d["wo"] = nc.dram_tensor("wo", [P, c.JC, c.E], BF16, kind="ExternalInput").ap()
    d["w1"] = nc.dram_tensor("w1", [P, c.KE, c.F], BF16, kind="ExternalInput").ap()
    d["w2"] = nc.dram_tensor("w2", [P, c.KF, c.E], BF16, kind="ExternalInput").ap()
    for nm, cols in [
        ("ln1g", c.KE), ("ln1b", c.KE), ("ln2g", c.KE), ("ln2b", c.KE),
        ("boc", c.KE), ("mb1", c.KF), ("mb2", c.KE),
    ]:
        d[nm] = nc.dram_tensor(nm, [P, cols], F32, kind="ExternalInput").ap()
    d["mask01"] = nc.dram_tensor(
        "mask01", [P, 4 * c.Tq], BF16, kind="ExternalInput"
    ).ap()
    d["ident"] = nc.dram_tensor("ident", [P, P], BF16, kind="ExternalInput").ap()
    d["sel64"] = nc.dram_tensor("sel64", [4, 4 * 64], BF16, kind="ExternalInput").ap()
    d["out_t"] = nc.dram_tensor("out_t", [c.E, c.Tq], F32, kind="ExternalOutput").ap()

    with tile.TileContext(nc) as tc:
        _emit(tc, c, d)
    nc.compile()
    return nc


# --------------------------------------------------------------------------
# host side
# --------------------------------------------------------------------------
def _pack_lhsT(w, cols_per_block):
    """[R, C] -> [128, R//128, C] lhsT layout (contiguous DMA)."""
    R, C = w.shape
    assert R % P == 0 and C == cols_per_block
    return np.ascontiguousarray(
        w.reshape(R // P, P, C).transpose(1, 0, 2)
    )


def shard_inputs(inputs, c: Cfg = CFG):
    x = np.ascontiguousarray(np.asarray(inputs["x"], np.float32))
    bf = lambda a: np.ascontiguousarray(np.asarray(a, np.float32)).astype(NPBF16)

    chunks = lambda v, k: np.ascontiguousarray(
        np.asarray(v, np.float32).reshape(k, P).T
    )
    com = {
        "w1": _pack_lhsT(bf(inputs["W1"]), c.F),
        "w2": _pack_lhsT(bf(inputs["W2"]), c.E),
        "ln1g": chunks(inputs["ln1_g"], c.KE),
        "ln1b": chunks(inputs["ln1_b"], c.KE),
        "ln2g": chunks(inputs["ln2_g"], c.KE),
        "ln2b": chunks(inputs["ln2_b"], c.KE),
        "boc": chunks(inputs["bo"], c.KE),
        "mb1": chunks(inputs["b1"], c.KF),
        "mb2": chunks(inputs["b2"], c.KE),
    }

    p_idx = np.arange(P)[:, None]
    tq_idx = np.arange(c.Tq)[None, :]
    mask = np.zeros((P, 4 * c.Tq), np.float32)
    for jb in range(4):
        mask[:, jb * c.Tq : (jb + 1) * c.Tq] = np.where(
            tq_idx >= (jb * P + p_idx), 0.0, -1.0e9
        )
    com["mask01"] = mask.astype(NPBF16)
    com["ident"] = np.eye(P, dtype=np.float32).astype(NPBF16)

    sel = np.zeros((4, 4 * 64), np.float32)
    for h in range(4):
        sel[h, h * 64 : (h + 1) * 64] = 1.0
    com["sel64"] = sel.astype(NPBF16)

    Wq, Wk, Wv = (bf(inputs[k]) for k in ("Wq", "Wk", "Wv"))
    Wo = bf(inputs["Wo"])
    maps = []
    for core in range(c.NC):
        b, r = core // c.CPB, core % c.CPB
        hs = slice(r * 2 * P, (r + 1) * 2 * P)  # this core's 256 head-features
        tok = np.concatenate(
            [np.arange(qc * c.Tq + r * P, qc * c.Tq + r * P + P) for qc in range(c.NQC)]
        )
        m = dict(com)
        m["xt"] = np.ascontiguousarray(x[b].T).astype(NPBF16)
        m["x_own"] = np.ascontiguousarray(x[b][tok, :].T)
        m["wq"] = _pack_lhsT(Wq[:, hs], 2 * P)
        m["wk"] = _pack_lhsT(Wk[:, hs], 2 * P)
        m["wv"] = _pack_lhsT(Wv[:, hs], 2 * P)
        m["wo"] = _pack_lhsT(np.ascontiguousarray(Wo[hs, :]), c.E)
        maps.append(m)
    return maps


def assemble(results, c: Cfg = CFG):
    out = np.empty((c.B, c.T, c.E), np.float32)
    for core in range(c.NC):
        b, r = core // c.CPB, core % c.CPB
        tok = np.concatenate(
            [np.arange(qc * c.Tq + r * P, qc * c.Tq + r * P + P) for qc in range(c.NQC)]
        )
        out[b, tok, :] = results[core]["out_t"].T
    return out


_NC_CACHE = {}


def _get_nc(c: Cfg = CFG):
    if c not in _NC_CACHE:
        _NC_CACHE[c] = build_program(c)
    return _NC_CACHE[c]


LAST_RESULT = None


def kernel(**inputs):
    global LAST_RESULT
    c = CFG
    nc = _get_nc(c)
    maps = shard_inputs(inputs, c)
    res = bass_utils.run_bass_kernel_spmd(nc, maps, core_ids=list(range(c.NC)))
    LAST_RESULT = res
    return assemble(res.results, c)


# revision 43
# speedup vs baseline: 1.0114x; 1.0114x over previous
"""Trainium2 Bass kernel for a pre-norm transformer block (dense_transformer).

Computation (per reference):
    x = x + Attn(LN1(x));  x = x + MLP(LN2(x))
with causal multi-head attention (H=16 heads, D=64) and a 4E ReLU MLP.

Sharding: DP-2 on batch x TP-4 on heads.  Core c = b*4 + r computes
LN1(x_b) over all T tokens, Q/K/V + causal attention for heads 4r..4r+3
only (so K/V projections are not recomputed 4x and score tiles above the
causal diagonal are skipped entirely), then the partial out-projection
for all tokens.  A single bf16 ReduceScatter over each 4-core group sums
the head-group partials and hands core r its 512-token slice, on which it
runs residual + LN2 + the full 4E MLP.

Layouts are feature-major throughout (E on partitions, tokens on the free
axis).  Softmax row sums come free from a ones-column appended to V
(M=65 attnV matmuls).  All weights are host-packed into lhsT layout so
every DMA is contiguous.  Matmuls run in bf16 with fp32 PSUM.
"""

from dataclasses import dataclass

import numpy as np
import ml_dtypes

import concourse.bass as bass  # noqa: F401
import concourse.mybir as mybir
import concourse.tile as tile
from concourse import bacc
from concourse import bass_utils

F32 = mybir.dt.float32
BF16 = mybir.dt.bfloat16
AF = mybir.ActivationFunctionType
OP = mybir.AluOpType
NPBF16 = ml_dtypes.bfloat16

P = 128


@dataclass(frozen=True)
class Cfg:
    B: int = 2
    T: int = 2048
    E: int = 1024
    H: int = 16
    D: int = 64
    NC: int = 8
    eps: float = 1e-5

    @property
    def CPB(self):  # cores per batch (TP group size)
        return self.NC // self.B

    @property
    def Tq(self):  # tokens owned per core (MLP stage)
        return self.T // self.CPB

    @property
    def KE(self):  # E / 128
        return self.E // P

    @property
    def TK(self):  # T / 128 context tiles
        return self.T // P

    @property
    def HPC(self):  # heads per core
        return self.H // self.CPB

    @property
    def JC(self):  # 128-row head-pair blocks per core
        return self.HPC // 2

    @property
    def F(self):
        return 4 * self.E

    @property
    def KF(self):
        return self.F // P

    @property
    def NQC(self):  # query chunks of Tq
        return self.T // self.Tq

    def check(self):
        assert self.D == 64 and self.E == self.H * self.D
        assert self.Tq == 512 and self.HPC == 4 and self.JC == 2
        assert self.T % P == 0 and self.E % P == 0 and self.F % P == 0


CFG = Cfg()


class Pools:
    """Tile pools with explicit open/close (LIFO per side, per space)."""

    def __init__(self, tc, prefix=""):
        self.tc = tc
        self.prefix = prefix
        self.live = {}

    def open(self, key, bufs, space=None, side=None):
        kw = dict(name=self.prefix + key, bufs=bufs)
        if space:
            kw["space"] = space
        if side:
            kw["side"] = side
        cm = self.tc.tile_pool(**kw)
        pool = cm.__enter__()
        self.live[key] = cm
        return pool

    def close(self, *keys):
        for key in keys:
            self.live.pop(key).__exit__(None, None, None)

    def close_all(self):
        for key in reversed(list(self.live)):
            self.close(key)


def _emit(tc, c: Cfg, d):
    nc = tc.nc
    E, T, Tq = c.E, c.T, c.Tq
    KE, TK, KF, JC, NQC, HPC = c.KE, c.TK, c.KF, c.JC, c.NQC, c.HPC
    DV = 65  # V cols per head incl. ones column
    SCL = 1.0 / float(np.sqrt(c.D))

    pp = Pools(tc)

    # ---------------- constants (whole-kernel lifetime) --------------------
    const = pp.open("const", 1)
    ones_bf = const.tile([P, 1], BF16, name="ones_bf")
    nc.vector.memset(ones_bf[:], 1.0)
    sel64 = const.tile([4, 4 * 64], BF16, name="sel64")
    nc.sync.dma_start(sel64[:], d["sel64"])
    gbt = {}
    for nm, cols in [
        ("ln1g", KE), ("ln1b", KE), ("ln2g", KE), ("ln2b", KE),
        ("boc", KE), ("mb1", KF), ("mb2", KE),
    ]:
        gbt[nm] = const.tile([P, cols], F32, name=nm + "_t")
        nc.sync.dma_start(gbt[nm][:], d[nm])

    ident = const.tile([P, P], BF16, name="ident")
    nc.sync.dma_start(ident[:], d["ident"])
    wrm = const.tile([P, Tq], BF16, name="wrm")
    nc.vector.memset(wrm[:], 0.0)

    p_band = pp.open("bandp", 1)
    mask01 = p_band.tile([P, 4 * Tq], BF16, name="mask01")
    nc.sync.dma_start(mask01[:], d["mask01"])

    # ---------------- DRAM bounce buffers for the ReduceScatters -----------
    # one RS per query chunk; rank r of each 4-core group receives token
    # stripe r (128 tokens) of that chunk, so the collectives pipeline under
    # the remaining attention compute.
    p_dram = pp.open("dram", 1, "DRAM")
    bin_q = [p_dram.tile([4 * E, P], BF16, name=f"bin{qc}") for qc in range(NQC)]
    bout_q = [p_dram.tile([E, P], BF16, name=f"bout{qc}") for qc in range(NQC)]

    # ---------------- PE warmup (pstate ramp) ------------------------------
    ps_wm = pp.open("warm_ps", 1, "PSUM")
    wmp = ps_wm.tile([1, Tq], F32, name="wmp")
    for _w in range(24):
        nc.tensor.matmul(wmp[:], ones_bf[:], wrm[:], start=True, stop=True)
    pp.close("warm_ps")

    # ======================================================================
    # Phase 0: load x^T (bf16) + LayerNorm1 over all T -> xn (bf16)
    # ======================================================================
    p_kq = pp.open("kqp", 1)
    p_vs = pp.open("vsp", 1)
    qt = [p_kq.tile([P, T], BF16, name=f"qt{j}") for j in range(JC)]
    kt = [p_kq.tile([P, T], BF16, name=f"kt{j}") for j in range(JC)]
    vsb = p_vs.tile([P, TK, HPC, DV], BF16, name="vsb")
    nc.vector.memset(vsb[:, :, :, DV - 1 : DV], 1.0)

    p_xn = pp.open("xnp", 1)
    p_w3 = pp.open("w3p", 1)
    wqt = p_w3.tile([P, KE, 2 * P], BF16, name="wqt")
    wkt = p_w3.tile([P, KE, 2 * P], BF16, name="wkt")
    wvt = p_w3.tile([P, KE, 2 * P], BF16, name="wvt")
    nc.sync.dma_start(wkt[:], d["wk"])
    p_xt = pp.open("xtp", 1)
    xt = [p_xt.tile([P, T], BF16, name=f"xt{e}") for e in range(KE)]
    xn = [p_xn.tile([P, T], BF16, name=f"xn{e}") for e in range(KE)]
    for e in range(KE):
        nc.sync.dma_start(xt[e][:], d["xt"][e * P : (e + 1) * P, :])
    nc.sync.dma_start(wqt[:], d["wq"])
    nc.sync.dma_start(wvt[:], d["wv"])

    ps_qkv = pp.open("qkv_ps", 2, "PSUM")
    p_tmp = pp.open("ln_tmp", 4)
    p_rows = pp.open("ln_rows", 1)
    p_bcs = pp.open("ln_bcs", 2)
    ps_st = pp.open("ln_st", 1, "PSUM")

    def kq_proj(ci):
        cs = slice(ci * Tq, (ci + 1) * Tq)
        for j in range(JC):
            for w_t, dst in ((wkt, kt), (wqt, qt)):
                ps = ps_qkv.tile([P, Tq], F32, name="kq_ps")
                for e in range(KE):
                    nc.tensor.matmul(
                        ps[:], w_t[:, e, j * P : (j + 1) * P], xn[e][:, cs],
                        start=(e == 0), stop=(e == KE - 1),
                    )
                nc.scalar.copy(dst[j][:, cs], ps[:])

    for ci in range(NQC):
        cs = slice(ci * Tq, (ci + 1) * Tq)
        s1 = ps_st.tile([1, Tq], F32, name="s1")
        s2 = ps_st.tile([1, Tq], F32, name="s2")
        for e in range(KE):
            x2 = p_tmp.tile([P, Tq], BF16, name="x2bf")
            nc.scalar.square(x2[:], xt[e][:, cs])
            nc.tensor.matmul(s1[:], ones_bf[:], xt[e][:, cs], start=(e == 0), stop=(e == KE - 1))
            nc.tensor.matmul(s2[:], ones_bf[:], x2[:], start=(e == 0), stop=(e == KE - 1))
        mu = p_rows.tile([1, Tq], F32, name="mu")
        nc.vector.tensor_scalar_mul(mu[:], s1[:], 1.0 / E)
        ve = p_rows.tile([1, Tq], F32, name="ve")
        nc.vector.tensor_scalar(ve[:], s2[:], 1.0 / E, c.eps, OP.mult, OP.add)
        mu2 = p_rows.tile([1, Tq], F32, name="mu2")
        nc.vector.tensor_tensor(mu2[:], mu[:], mu[:], OP.mult)
        vee = p_rows.tile([1, Tq], F32, name="vee")
        nc.vector.tensor_tensor(vee[:], ve[:], mu2[:], OP.subtract)
        lv = p_rows.tile([1, Tq], F32, name="lv")
        nc.scalar.activation(lv[:], vee[:], AF.Ln)
        rstd = p_rows.tile([1, Tq], F32, name="rstd")
        nc.scalar.activation(rstd[:], lv[:], AF.Exp, scale=-0.5)

        mub = p_bcs.tile([P, Tq], F32, name="mub")
        nc.gpsimd.partition_broadcast(mub[:], mu[:])
        rsb = p_bcs.tile([P, Tq], F32, name="rsb")
        nc.gpsimd.partition_broadcast(rsb[:], rstd[:])

        for e in range(KE):
            # alternate DVE / gpsimd so the normalize stream isn't one-engine
            eng = nc.vector if e % 2 == 0 else nc.gpsimd
            t1 = p_tmp.tile([P, Tq], F32, name=f"t1_{e % 2}")
            eng.tensor_tensor(t1[:], xt[e][:, cs], mub[:], OP.subtract)
            t2 = p_tmp.tile([P, Tq], F32, name=f"t2_{e % 2}")
            eng.tensor_tensor(t2[:], t1[:], rsb[:], OP.mult)
            eng.tensor_scalar(
                xn[e][:, cs], t2[:],
                gbt["ln1g"][:, e : e + 1], gbt["ln1b"][:, e : e + 1],
                OP.mult, OP.add,
            )
        if ci >= 1:
            kq_proj(ci - 1)
    kq_proj(NQC - 1)
    pp.close("ln_st", "ln_bcs", "ln_rows", "ln_tmp", "xtp")

    # ======================================================================
    # Phase 1: V projection (K/Q were interleaved with LN1 above)
    # ======================================================================
    for t in range(TK):
        ps = ps_qkv.tile([P, 2 * P], F32, name="v_ps")
        for e in range(KE):
            nc.tensor.matmul(
                ps[:], xn[e][:, t * P : (t + 1) * P], wvt[:, e, :],
                start=(e == 0), stop=(e == KE - 1),
            )
        nc.scalar.copy(
            vsb[:, t, :, 0:64],
            ps[:].rearrange("p (h v) -> p h v", h=HPC),
        )
    pp.close("qkv_ps", "w3p", "xnp")

    # prefetch the out-proj + MLP weights + residual slice while attention runs
    p_wo = pp.open("wop", 1, side="right")
    wot = p_wo.tile([P, JC, E], BF16, name="wot")
    nc.scalar.dma_start(wot[:], d["wo"])
    p_w12 = pp.open("w12p", 1, side="right")
    w1t = p_w12.tile([P, KE, c.F], BF16, name="w1t")
    w2t = p_w12.tile([P, KF, E], BF16, name="w2t")
    # w1/w2 are loaded in 1MB chunks interleaved into the attention stream
    # (see finalize) so they never monopolize the DMA engines or queues
    wload = []
    for e in range(KE):
        wload.append((w1t[:, e, :], d["w1"][:, e, :]))
    for fb in range(KE):
        wload.append((
            w2t[:, 4 * fb : 4 * (fb + 1), :], d["w2"][:, 4 * fb : 4 * (fb + 1), :]
        ))
    # ======================================================================
    # Phase 2: causal attention for 4 heads (2 pairs), all query chunks
    # ======================================================================
    p_ao = pp.open("aop", 1)
    p_pr = pp.open("probs", 4)
    p_rst = pp.open("rsst", 1)
    p_st2 = pp.open("rstage", 2)
    ps_av = pp.open("av_ps", 1, "PSUM")
    ps_ss = pp.open("ss_ps", 2, "PSUM")

    all_units = [
        (qc, t, p) for qc in range(NQC) for t in range(4 * qc + 4) for p in range(JC)
    ]
    LOOK = 2  # score-lookahead units (PSUM ring self-regulates via WAR)
    avp = {}

    def emit_ss(i):
        qc, t, p = all_units[i]
        jb = t - 4 * qc  # >= 0 on the causal diagonal band
        qs = slice(qc * Tq, (qc + 1) * Tq)
        ssu = ps_ss.tile([P, 2 * Tq], F32, name="ss")
        for s in (0, 1):
            nc.tensor.matmul(
                ssu[:, s * Tq : (s + 1) * Tq],
                kt[p][s * 64 : (s + 1) * 64, t * P : (t + 1) * P],
                qt[p][s * 64 : (s + 1) * 64, qs],
                start=True, stop=(jb < 0),
                tile_position=(s * 64, 0),
                skip_group_check=True,
            )
        if jb >= 0:
            # add -1e9 above the diagonal straight into the score PSUM
            for s in (0, 1):
                nc.tensor.matmul(
                    ssu[:, s * Tq : (s + 1) * Tq], ident[:],
                    mask01[:, jb * Tq : (jb + 1) * Tq],
                    start=False, stop=True,
                    skip_group_check=True,
                )
        return ssu

    aop_all = {}

    def finalize_part1(qc):
        """Copy the accumulators out of PSUM right away to unblock the next
        chunk's avp matmuls; the rest of the finalize is emitted a few units
        into the next chunk so its latency hides under the running stream."""
        avs = p_rst.tile([P, HPC * Tq], F32, name="avs")
        for h in range(HPC):
            nc.vector.tensor_copy(avs[0:DV, h * Tq : (h + 1) * Tq], avp[h][:])
        return avs

    def finalize_part2a(qc, avs):
        rs4 = p_st2.tile([4, Tq], F32, name="rs4", bufs=1)
        nc.sync.dma_start(rs4[:], avs[64:65, :])
        lrs = p_st2.tile([4, Tq], F32, name="lrs", bufs=1)
        nc.scalar.activation(lrs[:], rs4[:], AF.Ln)
        irs = p_st2.tile([4, Tq], BF16, name="irs", bufs=1)
        nc.scalar.activation(irs[:], lrs[:], AF.Exp, scale=-1.0)

        aop = [p_ao.tile([P, Tq], BF16, name=f"aop{qc}_{p}") for p in range(JC)]
        for p in range(JC):
            nb = ps_ss.tile([P, 2 * Tq], F32, name="ss")
            for s in (0, 1):
                nc.tensor.matmul(
                    nb[0:64, s * Tq : (s + 1) * Tq],
                    sel64[:, (2 * p + s) * 64 : (2 * p + s + 1) * 64],
                    irs[:],
                    start=True, stop=True,
                    skip_group_check=True,
                )
            nbs = p_st2.tile([64, 2 * Tq], BF16, name="nbs", bufs=1)
            nc.vector.tensor_copy(nbs[:], nb[0:64, :])
            nc.gpsimd.tensor_tensor(
                aop[p][0:64, :], avs[0:64, 2 * p * Tq : (2 * p + 1) * Tq],
                nbs[:, 0:Tq], OP.mult,
            )
            ost = p_st2.tile([64, Tq], BF16, name="ost")
            nc.gpsimd.tensor_tensor(
                ost[:], avs[0:64, (2 * p + 1) * Tq : (2 * p + 2) * Tq],
                nbs[:, Tq : 2 * Tq], OP.mult,
            )
            nc.sync.dma_start(aop[p][64:128, :], ost[:])
        return aop

    def finalize_part2b(qc, aop, last):
        obq = p_st2.tile([P, KE, Tq], BF16, name="obq", bufs=1)
        for e in range(KE):
            po = ps_ss.tile([P, 2 * Tq], F32, name="ss")
            for p in range(JC):
                nc.tensor.matmul(
                    po[:, 0:Tq], wot[:, p, e * P : (e + 1) * P], aop[p][:],
                    start=(p == 0), stop=(p == JC - 1),
                    skip_group_check=True,
                )
            if last and e % 2 == 1:
                nc.scalar.copy(obq[:, e, :], po[:, 0:Tq])
            else:
                nc.vector.tensor_copy(obq[:, e, :], po[:, 0:Tq])
        for j in range(4):
            nc.sync.dma_start(
                bin_q[qc][j * E : (j + 1) * E, :].rearrange("(e p) c -> p e c", p=P),
                obq[:, :, j * P : (j + 1) * P],
            )
        nc.gpsimd.collective_compute(
            "ReduceScatter",
            OP.add,
            replica_groups=[[0, 1, 2, 3], [4, 5, 6, 7]],
            ins=[bin_q[qc].opt()],
            outs=[bout_q[qc].opt()],
        )

        for _ in range(4):
            if wload:
                dst, src = wload.pop(0)
                nc.sync.dma_start(dst, src)

    ss_ring = [emit_ss(i) for i in range(min(LOOK, len(all_units)))]
    pend_a = None  # (qc, avs, waited)
    pend_b = None  # (qc, aop, waited)
    for i, (qc, t, p) in enumerate(all_units):
        ntile = 4 * qc + 4
        if t == 0 and p == 0:
            for h in range(HPC):
                avp[h] = ps_av.tile([DV, Tq], F32, name=f"avp{h}")
        pr = p_pr.tile([P, 2 * Tq], BF16, name="pr")
        nc.scalar.activation(pr[:], ss_ring[0][:], AF.Exp, scale=SCL)
        ss_ring.pop(0)
        if i + LOOK < len(all_units):
            ss_ring.append(emit_ss(i + LOOK))
        for s in (0, 1):
            h = 2 * p + s
            nc.tensor.matmul(
                avp[h][:],
                vsb[:, t, h, :],
                pr[:, s * Tq : (s + 1) * Tq],
                start=(t == 0), stop=(t == ntile - 1),
                skip_group_check=True,
            )
        if pend_a is not None:
            fqc, favs, waited = pend_a
            if waited >= 2:
                pend_b = (fqc, finalize_part2a(fqc, favs), 0)
                pend_a = None
            else:
                pend_a = (fqc, favs, waited + 1)
        elif pend_b is not None:
            fqc, faop, waited = pend_b
            if waited >= 4:
                finalize_part2b(fqc, faop, last=False)
                pend_b = None
            else:
                pend_b = (fqc, faop, waited + 1)
        if t == ntile - 1 and p == JC - 1:
            avs = finalize_part1(qc)
            if i == len(all_units) - 1:
                if pend_b is not None:
                    finalize_part2b(pend_b[0], pend_b[1], last=False)
                    pend_b = None
                finalize_part2b(qc, finalize_part2a(qc, avs), last=True)
            else:
                pend_a = (qc, avs, 0)
    if pend_b is not None:
        finalize_part2b(pend_b[0], pend_b[1], last=True)
        pend_b = None

    pp.close("ss_ps", "av_ps", "rstage", "rsst", "probs", "aop")
    pp.close("vsp", "kqp", "bandp")

    # ======================================================================
    # Phase 3: ReduceScatter partials; residual + bo -> xres; LN2 -> xn2
    # ======================================================================
    p_xo = pp.open("xop", 1, side="right")
    xown = [p_xo.tile([P, Tq], F32, name=f"xown{e}") for e in range(KE)]
    for e in range(KE):
        nc.sync.dma_start(xown[e][:], d["x_own"][e * P : (e + 1) * P, :])

    p_xr = pp.open("xrp", 1)
    p_x2 = pp.open("xn2p", 1)
    p_att = pp.open("attp", 1)
    xres = [p_xr.tile([P, Tq], F32, name=f"xres{e}") for e in range(KE)]
    xn2 = [p_x2.tile([P, Tq], BF16, name=f"xn2{e}") for e in range(KE)]
    att = p_att.tile([P, KE, Tq], BF16, name="att")
    for qc in range(NQC):
        nc.sync.dma_start(
            att[:, :, qc * P : (qc + 1) * P],
            bout_q[qc].rearrange("(e p) c -> p e c", p=P),
        )
    # stripe-major: stripes 0-2 arrived long before the last ReduceScatter,
    # so their residual adds run while it is still in flight
    for qc in range(NQC):
        sl_ = slice(qc * P, (qc + 1) * P)
        for e in range(KE):
            nc.vector.scalar_tensor_tensor(
                xres[e][:, sl_], att[:, e, sl_], gbt["boc"][:, e : e + 1],
                xown[e][:, sl_], OP.add, OP.add,
            )
    pp.close("attp")
    pp.close("xop")

    p_tmp = pp.open("ln2_tmp", 3)
    p_rows = pp.open("ln2_rows", 1)
    p_bc2 = pp.open("ln2_bcs", 1)
    ps_st = pp.open("ln2_st", 1, "PSUM")
    s1 = ps_st.tile([1, Tq], F32, name="s1b")
    s2 = ps_st.tile([1, Tq], F32, name="s2b")
    for e in range(KE):
        xbf = p_tmp.tile([P, Tq], BF16, name="xbf2")
        nc.vector.tensor_copy(xbf[:], xres[e][:])
        x2 = p_tmp.tile([P, Tq], BF16, name="x2bf2")
        nc.scalar.square(x2[:], xres[e][:])
        nc.tensor.matmul(s1[:], ones_bf[:], xbf[:], start=(e == 0), stop=(e == KE - 1))
        nc.tensor.matmul(s2[:], ones_bf[:], x2[:], start=(e == 0), stop=(e == KE - 1))
    mu = p_rows.tile([1, Tq], F32, name="mu_2")
    nc.vector.tensor_scalar_mul(mu[:], s1[:], 1.0 / E)
    ve = p_rows.tile([1, Tq], F32, name="ve_2")
    nc.vector.tensor_scalar(ve[:], s2[:], 1.0 / E, c.eps, OP.mult, OP.add)
    mu2 = p_rows.tile([1, Tq], F32, name="mu2_2")
    nc.vector.tensor_tensor(mu2[:], mu[:], mu[:], OP.mult)
    vee = p_rows.tile([1, Tq], F32, name="vee_2")
    nc.vector.tensor_tensor(vee[:], ve[:], mu2[:], OP.subtract)
    lv = p_rows.tile([1, Tq], F32, name="lv_2")
    nc.scalar.activation(lv[:], vee[:], AF.Ln)
    rstd = p_rows.tile([1, Tq], F32, name="rstd_2")
    nc.scalar.activation(rstd[:], lv[:], AF.Exp, scale=-0.5)
    mub = p_bc2.tile([P, Tq], F32, name="mub2")
    nc.gpsimd.partition_broadcast(mub[:], mu[:])
    rsb = p_bc2.tile([P, Tq], F32, name="rsb2")
    nc.gpsimd.partition_broadcast(rsb[:], rstd[:])
    for e in range(KE):
        eng = nc.vector if e % 2 == 0 else nc.gpsimd
        t1 = p_tmp.tile([P, Tq], F32, name=f"t1b_{e % 2}")
        eng.tensor_tensor(t1[:], xres[e][:], mub[:], OP.subtract)
        t2 = p_tmp.tile([P, Tq], F32, name=f"t2b_{e % 2}")
        eng.tensor_tensor(t2[:], t1[:], rsb[:], OP.mult)
        eng.tensor_scalar(
            xn2[e][:], t2[:],
            gbt["ln2g"][:, e : e + 1], gbt["ln2b"][:, e : e + 1],
            OP.mult, OP.add,
        )
    pp.close("ln2_st", "ln2_bcs", "ln2_rows", "ln2_tmp")

    # ======================================================================
    # Phase 4: MLP (layer 1 streamed with first 6 e-tiles of layer 2)
    # ======================================================================
    EH = min(KE, 6)
    p_h1 = pp.open("h1p", 1, side="right")
    p_out = pp.open("outp", 2)
    ps_h1 = pp.open("h1_ps", 2, "PSUM")
    ps_h2a = pp.open("h2a_ps", 1, "PSUM")

    h1 = [p_h1.tile([P, Tq], BF16, name=f"h1{f}") for f in range(KF)]
    h2a = [ps_h2a.tile([P, Tq], F32, name=f"h2a{e}") for e in range(EH)]
    for f in range(KF):
        ps = ps_h1.tile([P, Tq], F32, name="h1ps")
        for e in range(KE):
            nc.tensor.matmul(
                ps[:], w1t[:, e, f * P : (f + 1) * P], xn2[e][:],
                start=(e == 0), stop=(e == KE - 1),
            )
        nc.scalar.activation(
            h1[f][:], ps[:], AF.Relu, bias=gbt["mb1"][:, f : f + 1], scale=1.0
        )
        for e in range(EH):
            nc.tensor.matmul(
                h2a[e][:], w2t[:, f, e * P : (e + 1) * P], h1[f][:],
                start=(f == 0), stop=(f == KF - 1),
            )
    for e in range(EH):
        of = p_out.tile([P, Tq], F32, name="outf")
        nc.vector.scalar_tensor_tensor(
            of[:], h2a[e][:], gbt["mb2"][:, e : e + 1], xres[e][:], OP.add, OP.add
        )
        nc.sync.dma_start(d["out_t"][e * P : (e + 1) * P, :], of[:])
    pp.close("h2a_ps", "h1_ps")

    if EH < KE:
        ps_h2b = pp.open("h2b_ps", 1, "PSUM")
        h2b = [ps_h2b.tile([P, Tq], F32, name=f"h2b{e}") for e in range(KE - EH)]
        for f in range(KF):
            for i, e in enumerate(range(EH, KE)):
                nc.tensor.matmul(
                    h2b[i][:], w2t[:, f, e * P : (e + 1) * P], h1[f][:],
                    start=(f == 0), stop=(f == KF - 1),
                )
        for i, e in enumerate(range(EH, KE)):
            of = p_out.tile([P, Tq], F32, name="outf")
            nc.vector.scalar_tensor_tensor(
                of[:], h2b[i][:], gbt["mb2"][:, e : e + 1], xres[e][:], OP.add, OP.add
            )
            nc.sync.dma_start(d["out_t"][e * P : (e + 1) * P, :], of[:])

    pp.close_all()


def build_program(c: Cfg = CFG):
    c.check()
    nc = bacc.Bacc(
        "TRN2",
        target_bir_lowering=False,
        debug=False,
        enable_asserts=False,
        num_devices=c.NC,
    )
    d = {}
    d["xt"] = nc.dram_tensor("xt", [c.E, c.T], BF16, kind="ExternalInput").ap()
    d["x_own"] = nc.dram_tensor("x_own", [c.E, c.Tq], F32, kind="ExternalInput").ap()
    d["wq"] = nc.dram_tensor("wq", [P, c.KE, 2 * P], BF16, kind="ExternalInput").ap()
    d["wk"] = nc.dram_tensor("wk", [P, c.KE, 2 * P], BF16, kind="ExternalInput").ap()
    d["wv"] = nc.dram_tensor("wv", [P, c.KE, 2 * P], BF16, kind="ExternalInput").ap()
    d["wo"] = nc.dram_tensor("wo", [P, c.JC, c.E], BF16, kind="ExternalInput").ap()
    d["w1"] = nc.dram_tensor("w1", [P, c.KE, c.F], BF16, kind="ExternalInput").ap()
    d["w2"] = nc.dram_tensor("w2", [P, c.KF, c.E], BF16, kind="ExternalInput").ap()
    for nm, cols in [
        ("ln1g", c.KE), ("ln1b", c.KE), ("ln2g", c.KE), ("ln2b", c.KE),
        ("boc", c.KE), ("mb1", c.KF), ("mb2", c.KE),
    ]:
        d[nm] = nc.dram_tensor(nm, [P, cols], F32, kind="ExternalInput").ap()
    d["mask01"] = nc.dram_tensor(
        "mask01", [P, 4 * c.Tq], BF16, kind="ExternalInput"
    ).ap()
    d["ident"] = nc.dram_tensor("ident", [P, P], BF16, kind="ExternalInput").ap()
    d["sel64"] = nc.dram_tensor("sel64", [4, 4 * 64], BF16, kind="ExternalInput").ap()
    d["out_t"] = nc.dram_tensor("out_t", [c.E, c.Tq], F32, kind="ExternalOutput").ap()

    with tile.TileContext(nc) as tc:
        _emit(tc, c, d)
    nc.compile()
    return nc


# --------------------------------------------------------------------------
# host side
# --------------------------------------------------------------------------
def _pack_lhsT(w, cols_per_block):
    """[R, C] -> [128, R//128, C] lhsT layout (contiguous DMA)."""
    R, C = w.shape
    assert R % P == 0 and C == cols_per_block
    return np.ascontiguousarray(
        w.reshape(R // P, P, C).transpose(1, 0, 2)
    )


def shard_inputs(inputs, c: Cfg = CFG):
    x = np.ascontiguousarray(np.asarray(inputs["x"], np.float32))
    bf = lambda a: np.ascontiguousarray(np.asarray(a, np.float32)).astype(NPBF16)

    chunks = lambda v, k: np.ascontiguousarray(
        np.asarray(v, np.float32).reshape(k, P).T
    )
    com = {
        "w1": _pack_lhsT(bf(inputs["W1"]), c.F),
        "w2": _pack_lhsT(bf(inputs["W2"]), c.E),
        "ln1g": chunks(inputs["ln1_g"], c.KE),
        "ln1b": chunks(inputs["ln1_b"], c.KE),
        "ln2g": chunks(inputs["ln2_g"], c.KE),
        "ln2b": chunks(inputs["ln2_b"], c.KE),
        "boc": chunks(inputs["bo"], c.KE),
        "mb1": chunks(inputs["b1"], c.KF),
        "mb2": chunks(inputs["b2"], c.KE),
    }

    p_idx = np.arange(P)[:, None]
    tq_idx = np.arange(c.Tq)[None, :]
    mask = np.zeros((P, 4 * c.Tq), np.float32)
    for jb in range(4):
        mask[:, jb * c.Tq : (jb + 1) * c.Tq] = np.where(
            tq_idx >= (jb * P + p_idx), 0.0, -1.0e9
        )
    com["mask01"] = mask.astype(NPBF16)
    com["ident"] = np.eye(P, dtype=np.float32).astype(NPBF16)

    sel = np.zeros((4, 4 * 64), np.float32)
    for h in range(4):
        sel[h, h * 64 : (h + 1) * 64] = 1.0
    com["sel64"] = sel.astype(NPBF16)

    Wq, Wk, Wv = (bf(inputs[k]) for k in ("Wq", "Wk", "Wv"))
    Wo = bf(inputs["Wo"])
    maps = []
    for core in range(c.NC):
        b, r = core // c.CPB, core % c.CPB
        hs = slice(r * 2 * P, (r + 1) * 2 * P)  # this core's 256 head-features
        tok = np.concatenate(
            [np.arange(qc * c.Tq + r * P, qc * c.Tq + r * P + P) for qc in range(c.NQC)]
        )
        m = dict(com)
        m["xt"] = np.ascontiguousarray(x[b].T).astype(NPBF16)
        m["x_own"] = np.ascontiguousarray(x[b][tok, :].T)
        m["wq"] = _pack_lhsT(Wq[:, hs], 2 * P)
        m["wk"] = _pack_lhsT(Wk[:, hs], 2 * P)
        m["wv"] = _pack_lhsT(Wv[:, hs], 2 * P)
        m["wo"] = _pack_lhsT(np.ascontiguousarray(Wo[hs, :]), c.E)
        maps.append(m)
    return maps


def assemble(results, c: Cfg = CFG):
    out = np.empty((c.B, c.T, c.E), np.float32)
    for core in range(c.NC):
        b, r = core // c.CPB, core % c.CPB
        tok = np.concatenate(
            [np.arange(qc * c.Tq + r * P, qc * c.Tq + r * P + P) for qc in range(c.NQC)]
        )
        out[b, tok, :] = results[core]["out_t"].T
    return out


_NC_CACHE = {}


def _get_nc(c: Cfg = CFG):
    if c not in _NC_CACHE:
        _NC_CACHE[c] = build_program(c)
    return _NC_CACHE[c]


LAST_RESULT = None


def kernel(**inputs):
    global LAST_RESULT
    c = CFG
    nc = _get_nc(c)
    maps = shard_inputs(inputs, c)
    res = bass_utils.run_bass_kernel_spmd(nc, maps, core_ids=list(range(c.NC)))
    LAST_RESULT = res
    return assemble(res.results, c)


# revision 45
# speedup vs baseline: 1.0169x; 1.0054x over previous
"""Trainium2 Bass kernel for a pre-norm transformer block (dense_transformer).

Computation (per reference):
    x = x + Attn(LN1(x));  x = x + MLP(LN2(x))
with causal multi-head attention (H=16 heads, D=64) and a 4E ReLU MLP.

Sharding: DP-2 on batch x TP-4 on heads.  Core c = b*4 + r computes
LN1(x_b) over all T tokens, Q/K/V + causal attention for heads 4r..4r+3
only (so K/V projections are not recomputed 4x and score tiles above the
causal diagonal are skipped entirely), then the partial out-projection
for all tokens.  A single bf16 ReduceScatter over each 4-core group sums
the head-group partials and hands core r its 512-token slice, on which it
runs residual + LN2 + the full 4E MLP.

Layouts are feature-major throughout (E on partitions, tokens on the free
axis).  Softmax row sums come free from a ones-column appended to V
(M=65 attnV matmuls).  All weights are host-packed into lhsT layout so
every DMA is contiguous.  Matmuls run in bf16 with fp32 PSUM.
"""

from dataclasses import dataclass

import numpy as np
import ml_dtypes

import concourse.bass as bass  # noqa: F401
import concourse.mybir as mybir
import concourse.tile as tile
from concourse import bacc
from concourse import bass_utils

F32 = mybir.dt.float32
BF16 = mybir.dt.bfloat16
AF = mybir.ActivationFunctionType
OP = mybir.AluOpType
NPBF16 = ml_dtypes.bfloat16

P = 128


@dataclass(frozen=True)
class Cfg:
    B: int = 2
    T: int = 2048
    E: int = 1024
    H: int = 16
    D: int = 64
    NC: int = 8
    eps: float = 1e-5

    @property
    def CPB(self):  # cores per batch (TP group size)
        return self.NC // self.B

    @property
    def Tq(self):  # tokens owned per core (MLP stage)
        return self.T // self.CPB

    @property
    def KE(self):  # E / 128
        return self.E // P

    @property
    def TK(self):  # T / 128 context tiles
        return self.T // P

    @property
    def HPC(self):  # heads per core
        return self.H // self.CPB

    @property
    def JC(self):  # 128-row head-pair blocks per core
        return self.HPC // 2

    @property
    def F(self):
        return 4 * self.E

    @property
    def KF(self):
        return self.F // P

    @property
    def NQC(self):  # query chunks of Tq
        return self.T // self.Tq

    def check(self):
        assert self.D == 64 and self.E == self.H * self.D
        assert self.Tq == 512 and self.HPC == 4 and self.JC == 2
        assert self.T % P == 0 and self.E % P == 0 and self.F % P == 0


CFG = Cfg()


class Pools:
    """Tile pools with explicit open/close (LIFO per side, per space)."""

    def __init__(self, tc, prefix=""):
        self.tc = tc
        self.prefix = prefix
        self.live = {}

    def open(self, key, bufs, space=None, side=None):
        kw = dict(name=self.prefix + key, bufs=bufs)
        if space:
            kw["space"] = space
        if side:
            kw["side"] = side
        cm = self.tc.tile_pool(**kw)
        pool = cm.__enter__()
        self.live[key] = cm
        return pool

    def close(self, *keys):
        for key in keys:
            self.live.pop(key).__exit__(None, None, None)

    def close_all(self):
        for key in reversed(list(self.live)):
            self.close(key)


def _emit(tc, c: Cfg, d):
    nc = tc.nc
    E, T, Tq = c.E, c.T, c.Tq
    KE, TK, KF, JC, NQC, HPC = c.KE, c.TK, c.KF, c.JC, c.NQC, c.HPC
    DV = 65  # V cols per head incl. ones column
    SCL = 1.0 / float(np.sqrt(c.D))

    pp = Pools(tc)

    # ---------------- constants (whole-kernel lifetime) --------------------
    const = pp.open("const", 1)
    ones_bf = const.tile([P, 1], BF16, name="ones_bf")
    nc.vector.memset(ones_bf[:], 1.0)
    sel64 = const.tile([4, 4 * 64], BF16, name="sel64")
    nc.sync.dma_start(sel64[:], d["sel64"])
    gbt = {}
    for nm, cols in [
        ("ln1g", KE), ("ln1b", KE), ("ln2g", KE), ("ln2b", KE),
        ("boc", KE), ("mb1", KF), ("mb2", KE),
    ]:
        gbt[nm] = const.tile([P, cols], F32, name=nm + "_t")
        nc.sync.dma_start(gbt[nm][:], d[nm])

    ident = const.tile([P, P], BF16, name="ident")
    nc.sync.dma_start(ident[:], d["ident"])
    wrm = const.tile([P, Tq], BF16, name="wrm")
    nc.vector.memset(wrm[:], 0.0)

    p_band = pp.open("bandp", 1)
    mask01 = p_band.tile([P, 4 * Tq], BF16, name="mask01")
    nc.sync.dma_start(mask01[:], d["mask01"])

    # ---------------- DRAM bounce buffers for the ReduceScatters -----------
    # one RS per query chunk; rank r of each 4-core group receives token
    # stripe r (128 tokens) of that chunk, so the collectives pipeline under
    # the remaining attention compute.
    p_dram = pp.open("dram", 1, "DRAM")
    bin_q = [p_dram.tile([4 * E, P], BF16, name=f"bin{qc}") for qc in range(NQC)]
    bout_q = [p_dram.tile([E, P], BF16, name=f"bout{qc}") for qc in range(NQC)]

    # ---------------- PE warmup (pstate ramp) ------------------------------
    ps_wm = pp.open("warm_ps", 1, "PSUM")
    wmp = ps_wm.tile([1, Tq], F32, name="wmp")
    for _w in range(24):
        nc.tensor.matmul(wmp[:], ones_bf[:], wrm[:], start=True, stop=True)
    pp.close("warm_ps")

    # ======================================================================
    # Phase 0: load x^T (bf16) + LayerNorm1 over all T -> xn (bf16)
    # ======================================================================
    p_kq = pp.open("kqp", 1)
    p_vs = pp.open("vsp", 1)
    qt = [p_kq.tile([P, T], BF16, name=f"qt{j}") for j in range(JC)]
    kt = [p_kq.tile([P, T], BF16, name=f"kt{j}") for j in range(JC)]
    vsb = p_vs.tile([P, TK, HPC, DV], BF16, name="vsb")
    nc.vector.memset(vsb[:, :, :, DV - 1 : DV], 1.0)

    p_xn = pp.open("xnp", 1)
    p_w3 = pp.open("w3p", 1)
    wqt = p_w3.tile([P, KE, 2 * P], BF16, name="wqt")
    wkt = p_w3.tile([P, KE, 2 * P], BF16, name="wkt")
    wvt = p_w3.tile([P, KE, 2 * P], BF16, name="wvt")
    nc.sync.dma_start(wkt[:], d["wk"])
    p_xt = pp.open("xtp", 1)
    xt = [p_xt.tile([P, T], BF16, name=f"xt{e}") for e in range(KE)]
    xn = [p_xn.tile([P, T], BF16, name=f"xn{e}") for e in range(KE)]
    for e in range(KE):
        nc.sync.dma_start(xt[e][:], d["xt"][e * P : (e + 1) * P, :])
    nc.sync.dma_start(wqt[:], d["wq"])
    nc.sync.dma_start(wvt[:], d["wv"])

    ps_qkv = pp.open("qkv_ps", 2, "PSUM")
    p_tmp = pp.open("ln_tmp", 4)
    p_rows = pp.open("ln_rows", 1)
    p_bcs = pp.open("ln_bcs", 2)
    ps_st = pp.open("ln_st", 1, "PSUM")

    def kq_proj(ci):
        cs = slice(ci * Tq, (ci + 1) * Tq)
        for j in range(JC):
            for w_t, dst in ((wkt, kt), (wqt, qt)):
                ps = ps_qkv.tile([P, Tq], F32, name="kq_ps")
                for e in range(KE):
                    nc.tensor.matmul(
                        ps[:], w_t[:, e, j * P : (j + 1) * P], xn[e][:, cs],
                        start=(e == 0), stop=(e == KE - 1),
                    )
                nc.scalar.copy(dst[j][:, cs], ps[:])

    for ci in range(NQC):
        cs = slice(ci * Tq, (ci + 1) * Tq)
        s1 = ps_st.tile([1, Tq], F32, name="s1")
        s2 = ps_st.tile([1, Tq], F32, name="s2")
        for e in range(KE):
            x2 = p_tmp.tile([P, Tq], BF16, name="x2bf")
            nc.scalar.square(x2[:], xt[e][:, cs])
            nc.tensor.matmul(s1[:], ones_bf[:], xt[e][:, cs], start=(e == 0), stop=(e == KE - 1))
            nc.tensor.matmul(s2[:], ones_bf[:], x2[:], start=(e == 0), stop=(e == KE - 1))
        mu = p_rows.tile([1, Tq], F32, name="mu")
        nc.vector.tensor_scalar_mul(mu[:], s1[:], 1.0 / E)
        ve = p_rows.tile([1, Tq], F32, name="ve")
        nc.vector.tensor_scalar(ve[:], s2[:], 1.0 / E, c.eps, OP.mult, OP.add)
        mu2 = p_rows.tile([1, Tq], F32, name="mu2")
        nc.vector.tensor_tensor(mu2[:], mu[:], mu[:], OP.mult)
        vee = p_rows.tile([1, Tq], F32, name="vee")
        nc.vector.tensor_tensor(vee[:], ve[:], mu2[:], OP.subtract)
        lv = p_rows.tile([1, Tq], F32, name="lv")
        nc.scalar.activation(lv[:], vee[:], AF.Ln)
        rstd = p_rows.tile([1, Tq], F32, name="rstd")
        nc.scalar.activation(rstd[:], lv[:], AF.Exp, scale=-0.5)

        mub = p_bcs.tile([P, Tq], F32, name="mub")
        nc.gpsimd.partition_broadcast(mub[:], mu[:])
        rsb = p_bcs.tile([P, Tq], F32, name="rsb")
        nc.gpsimd.partition_broadcast(rsb[:], rstd[:])

        for e in range(KE):
            # alternate DVE / gpsimd so the normalize stream isn't one-engine
            eng = nc.vector if e % 2 == 0 else nc.gpsimd
            t1 = p_tmp.tile([P, Tq], F32, name=f"t1_{e % 2}")
            eng.tensor_tensor(t1[:], xt[e][:, cs], mub[:], OP.subtract)
            t2 = p_tmp.tile([P, Tq], F32, name=f"t2_{e % 2}")
            eng.tensor_tensor(t2[:], t1[:], rsb[:], OP.mult)
            eng.tensor_scalar(
                xn[e][:, cs], t2[:],
                gbt["ln1g"][:, e : e + 1], gbt["ln1b"][:, e : e + 1],
                OP.mult, OP.add,
            )
        if ci >= 1:
            kq_proj(ci - 1)
    kq_proj(NQC - 1)
    pp.close("ln_st", "ln_bcs", "ln_rows", "ln_tmp", "xtp")

    # ======================================================================
    # Phase 1: V projection (K/Q were interleaved with LN1 above)
    # ======================================================================
    for t in range(TK):
        ps = ps_qkv.tile([P, 2 * P], F32, name="v_ps")
        for e in range(KE):
            nc.tensor.matmul(
                ps[:], xn[e][:, t * P : (t + 1) * P], wvt[:, e, :],
                start=(e == 0), stop=(e == KE - 1),
            )
        nc.scalar.copy(
            vsb[:, t, :, 0:64],
            ps[:].rearrange("p (h v) -> p h v", h=HPC),
        )
    pp.close("qkv_ps", "w3p", "xnp")

    # prefetch the out-proj + MLP weights + residual slice while attention runs
    p_wo = pp.open("wop", 1, side="right")
    wot = p_wo.tile([P, JC, E], BF16, name="wot")
    nc.scalar.dma_start(wot[:], d["wo"])
    p_w12 = pp.open("w12p", 1, side="right")
    w1t = p_w12.tile([P, KE, c.F], BF16, name="w1t")
    w2t = p_w12.tile([P, KF, E], BF16, name="w2t")
    # w1/w2 are loaded in 1MB chunks interleaved into the attention stream
    # (see finalize) so they never monopolize the DMA engines or queues
    wload = []
    for e in range(KE):
        wload.append((w1t[:, e, :], d["w1"][:, e, :]))
    for fb in range(KE):
        wload.append((
            w2t[:, 4 * fb : 4 * (fb + 1), :], d["w2"][:, 4 * fb : 4 * (fb + 1), :]
        ))
    # ======================================================================
    # Phase 2: causal attention for 4 heads (2 pairs), all query chunks
    # ======================================================================
    p_ao = pp.open("aop", 1)
    p_pr = pp.open("probs", 4)
    p_rst = pp.open("rsst", 1)
    p_st2 = pp.open("rstage", 2)
    ps_av = pp.open("av_ps", 1, "PSUM")
    ps_ss = pp.open("ss_ps", 2, "PSUM")

    all_units = [
        (qc, t, p) for qc in range(NQC) for t in range(4 * qc + 4) for p in range(JC)
    ]
    LOOK = 2  # score-lookahead units (PSUM ring self-regulates via WAR)
    avp = {}

    def emit_ss(i):
        qc, t, p = all_units[i]
        jb = t - 4 * qc  # >= 0 on the causal diagonal band
        qs = slice(qc * Tq, (qc + 1) * Tq)
        ssu = ps_ss.tile([P, 2 * Tq], F32, name="ss")
        for s in (0, 1):
            nc.tensor.matmul(
                ssu[:, s * Tq : (s + 1) * Tq],
                kt[p][s * 64 : (s + 1) * 64, t * P : (t + 1) * P],
                qt[p][s * 64 : (s + 1) * 64, qs],
                start=True, stop=(jb < 0),
                tile_position=(s * 64, 0),
                skip_group_check=True,
            )
        if jb >= 0:
            # add -1e9 above the diagonal straight into the score PSUM
            for s in (0, 1):
                nc.tensor.matmul(
                    ssu[:, s * Tq : (s + 1) * Tq], ident[:],
                    mask01[:, jb * Tq : (jb + 1) * Tq],
                    start=False, stop=True,
                    skip_group_check=True,
                )
        return ssu

    aop_all = {}

    def finalize_part1(qc):
        """Copy the accumulators out of PSUM right away to unblock the next
        chunk's avp matmuls; the rest of the finalize is emitted a few units
        into the next chunk so its latency hides under the running stream."""
        avs = p_rst.tile([P, HPC * Tq], F32, name="avs")
        for h in range(HPC):
            nc.vector.tensor_copy(avs[0:DV, h * Tq : (h + 1) * Tq], avp[h][:])
        return avs

    def finalize_part2a(qc, avs):
        rs4 = p_st2.tile([4, Tq], F32, name="rs4", bufs=1)
        nc.sync.dma_start(rs4[:], avs[64:65, :])
        lrs = p_st2.tile([4, Tq], F32, name="lrs", bufs=1)
        nc.scalar.activation(lrs[:], rs4[:], AF.Ln)
        irs = p_st2.tile([4, Tq], BF16, name="irs", bufs=1)
        nc.scalar.activation(irs[:], lrs[:], AF.Exp, scale=-1.0)

        aop = [p_ao.tile([P, Tq], BF16, name=f"aop{qc}_{p}") for p in range(JC)]
        for p in range(JC):
            nb = ps_ss.tile([P, 2 * Tq], F32, name="ss")
            for s in (0, 1):
                nc.tensor.matmul(
                    nb[0:64, s * Tq : (s + 1) * Tq],
                    sel64[:, (2 * p + s) * 64 : (2 * p + s + 1) * 64],
                    irs[:],
                    start=True, stop=True,
                    skip_group_check=True,
                )
            nbs = p_st2.tile([64, 2 * Tq], BF16, name="nbs", bufs=1)
            nc.vector.tensor_copy(nbs[:], nb[0:64, :])
            nc.gpsimd.tensor_tensor(
                aop[p][0:64, :], avs[0:64, 2 * p * Tq : (2 * p + 1) * Tq],
                nbs[:, 0:Tq], OP.mult,
            )
            ost = p_st2.tile([64, Tq], BF16, name="ost")
            nc.gpsimd.tensor_tensor(
                ost[:], avs[0:64, (2 * p + 1) * Tq : (2 * p + 2) * Tq],
                nbs[:, Tq : 2 * Tq], OP.mult,
            )
            nc.sync.dma_start(aop[p][64:128, :], ost[:])
        return aop

    def finalize_part2b(qc, aop, last):
        obq = p_st2.tile([P, KE, Tq], BF16, name="obq", bufs=1)
        for e in range(KE):
            po = ps_ss.tile([P, 2 * Tq], F32, name="ss")
            for p in range(JC):
                nc.tensor.matmul(
                    po[:, 0:Tq], wot[:, p, e * P : (e + 1) * P], aop[p][:],
                    start=(p == 0), stop=(p == JC - 1),
                    skip_group_check=True,
                )
            if last and e % 2 == 1:
                nc.scalar.copy(obq[:, e, :], po[:, 0:Tq])
            else:
                nc.vector.tensor_copy(obq[:, e, :], po[:, 0:Tq])
        for j in range(4):
            nc.sync.dma_start(
                bin_q[qc][j * E : (j + 1) * E, :].rearrange("(e p) c -> p e c", p=P),
                obq[:, :, j * P : (j + 1) * P],
            )
        nc.gpsimd.collective_compute(
            "ReduceScatter",
            OP.add,
            replica_groups=[[0, 1, 2, 3], [4, 5, 6, 7]],
            ins=[bin_q[qc].opt()],
            outs=[bout_q[qc].opt()],
        )

        for _ in range(4):
            if wload:
                dst, src = wload.pop(0)
                nc.sync.dma_start(dst, src)

    ss_ring = [emit_ss(i) for i in range(min(LOOK, len(all_units)))]
    pend_a = None  # (qc, avs, waited)
    pend_b = None  # (qc, aop, waited)
    for i, (qc, t, p) in enumerate(all_units):
        ntile = 4 * qc + 4
        if t == 0 and p == 0:
            for h in range(HPC):
                avp[h] = ps_av.tile([DV, Tq], F32, name=f"avp{h}")
        pr = p_pr.tile([P, 2 * Tq], BF16, name="pr")
        nc.scalar.activation(pr[:], ss_ring[0][:], AF.Exp, scale=SCL)
        ss_ring.pop(0)
        if i + LOOK < len(all_units):
            ss_ring.append(emit_ss(i + LOOK))
        for s in (0, 1):
            h = 2 * p + s
            nc.tensor.matmul(
                avp[h][:],
                vsb[:, t, h, :],
                pr[:, s * Tq : (s + 1) * Tq],
                start=(t == 0), stop=(t == ntile - 1),
                skip_group_check=True,
            )
        if pend_a is not None:
            fqc, favs, waited = pend_a
            if waited >= 2:
                pend_b = (fqc, finalize_part2a(fqc, favs), 0)
                pend_a = None
            else:
                pend_a = (fqc, favs, waited + 1)
        elif pend_b is not None:
            fqc, faop, waited = pend_b
            if waited >= 4:
                finalize_part2b(fqc, faop, last=False)
                pend_b = None
            else:
                pend_b = (fqc, faop, waited + 1)
        if t == ntile - 1 and p == JC - 1:
            avs = finalize_part1(qc)
            if i == len(all_units) - 1:
                if pend_b is not None:
                    finalize_part2b(pend_b[0], pend_b[1], last=False)
                    pend_b = None
                finalize_part2b(qc, finalize_part2a(qc, avs), last=True)
            else:
                pend_a = (qc, avs, 0)
    if pend_b is not None:
        finalize_part2b(pend_b[0], pend_b[1], last=True)
        pend_b = None

    pp.close("ss_ps", "av_ps", "rstage", "rsst", "probs", "aop")
    pp.close("vsp", "kqp", "bandp")

    # ======================================================================
    # Phase 3: ReduceScatter partials; residual + bo -> xres; LN2 -> xn2
    # ======================================================================
    p_xo = pp.open("xop", 1, side="right")
    xown = [p_xo.tile([P, Tq], F32, name=f"xown{e}") for e in range(KE)]
    for e in range(KE):
        nc.sync.dma_start(xown[e][:], d["x_own"][e * P : (e + 1) * P, :])

    p_xr = pp.open("xrp", 1)
    p_x2 = pp.open("xn2p", 1)
    p_att = pp.open("attp", 1)
    xres = [p_xr.tile([P, Tq], F32, name=f"xres{e}") for e in range(KE)]
    xn2 = [p_x2.tile([P, Tq], BF16, name=f"xn2{e}") for e in range(KE)]
    att = p_att.tile([P, KE, Tq], BF16, name="att")
    for qc in range(NQC):
        nc.sync.dma_start(
            att[:, :, qc * P : (qc + 1) * P],
            bout_q[qc].rearrange("(e p) c -> p e c", p=P),
        )
    # stripe-major: stripes 0-2 arrived long before the last ReduceScatter,
    # so their residual adds AND LN2 statistics run while it is in flight.
    # Each stripe accumulates into its own PSUM tile (clean start/stop groups).
    p_sq = pp.open("ln2_sq", 3)
    p_rows = pp.open("ln2_rows", 1)
    p_bc2 = pp.open("ln2_bcs", 1)
    ps_st = pp.open("ln2_st", 1, "PSUM")
    s1q = [ps_st.tile([1, P], F32, name=f"s1q{qc}") for qc in range(NQC)]
    s2q = [ps_st.tile([1, P], F32, name=f"s2q{qc}") for qc in range(NQC)]
    for qc in range(NQC):
        sl_ = slice(qc * P, (qc + 1) * P)
        for e in range(KE):
            nc.vector.scalar_tensor_tensor(
                xres[e][:, sl_], att[:, e, sl_], gbt["boc"][:, e : e + 1],
                xown[e][:, sl_], OP.add, OP.add,
            )
            xbf = p_sq.tile([P, P], BF16, name="xbf2")
            nc.vector.tensor_copy(xbf[:], xres[e][:, sl_])
            x2 = p_sq.tile([P, P], BF16, name="x2bf2")
            nc.scalar.square(x2[:], xres[e][:, sl_])
            nc.tensor.matmul(
                s1q[qc][:], ones_bf[:], xbf[:],
                start=(e == 0), stop=(e == KE - 1),
            )
            nc.tensor.matmul(
                s2q[qc][:], ones_bf[:], x2[:],
                start=(e == 0), stop=(e == KE - 1),
            )
    pp.close("xop")
    p_tmp = pp.open("ln2_tmp", 2)
    mu = p_rows.tile([1, Tq], F32, name="mu_2")
    ve = p_rows.tile([1, Tq], F32, name="ve_2")
    for qc in range(NQC):
        sl_ = slice(qc * P, (qc + 1) * P)
        nc.vector.tensor_scalar_mul(mu[0:1, sl_], s1q[qc][:], 1.0 / E)
        nc.vector.tensor_scalar(
            ve[0:1, sl_], s2q[qc][:], 1.0 / E, c.eps, OP.mult, OP.add
        )
    mu2 = p_rows.tile([1, Tq], F32, name="mu2_2")
    nc.vector.tensor_tensor(mu2[:], mu[:], mu[:], OP.mult)
    vee = p_rows.tile([1, Tq], F32, name="vee_2")
    nc.vector.tensor_tensor(vee[:], ve[:], mu2[:], OP.subtract)
    lv = p_rows.tile([1, Tq], F32, name="lv_2")
    nc.scalar.activation(lv[:], vee[:], AF.Ln)
    rstd = p_rows.tile([1, Tq], F32, name="rstd_2")
    nc.scalar.activation(rstd[:], lv[:], AF.Exp, scale=-0.5)
    mub = p_bc2.tile([P, Tq], F32, name="mub2")
    nc.gpsimd.partition_broadcast(mub[:], mu[:])
    rsb = p_bc2.tile([P, Tq], F32, name="rsb2")
    nc.gpsimd.partition_broadcast(rsb[:], rstd[:])
    for e in range(KE):
        eng = nc.vector if e % 2 == 0 else nc.gpsimd
        t1 = p_tmp.tile([P, Tq], F32, name=f"t1b_{e % 2}")
        eng.tensor_tensor(t1[:], xres[e][:], mub[:], OP.subtract)
        t2 = p_tmp.tile([P, Tq], F32, name=f"t2b_{e % 2}")
        eng.tensor_tensor(t2[:], t1[:], rsb[:], OP.mult)
        eng.tensor_scalar(
            xn2[e][:], t2[:],
            gbt["ln2g"][:, e : e + 1], gbt["ln2b"][:, e : e + 1],
            OP.mult, OP.add,
        )
    pp.close("ln2_tmp", "ln2_st", "ln2_bcs", "ln2_rows", "ln2_sq", "attp")

    # ======================================================================
    # Phase 4: MLP (layer 1 streamed with first 6 e-tiles of layer 2)
    # ======================================================================
    EH = min(KE, 6)
    p_h1 = pp.open("h1p", 1, side="right")
    p_out = pp.open("outp", 2)
    ps_h1 = pp.open("h1_ps", 2, "PSUM")
    ps_h2a = pp.open("h2a_ps", 1, "PSUM")

    h1 = [p_h1.tile([P, Tq], BF16, name=f"h1{f}") for f in range(KF)]
    h2a = [ps_h2a.tile([P, Tq], F32, name=f"h2a{e}") for e in range(EH)]
    for f in range(KF):
        ps = ps_h1.tile([P, Tq], F32, name="h1ps")
        for e in range(KE):
            nc.tensor.matmul(
                ps[:], w1t[:, e, f * P : (f + 1) * P], xn2[e][:],
                start=(e == 0), stop=(e == KE - 1),
            )
        nc.scalar.activation(
            h1[f][:], ps[:], AF.Relu, bias=gbt["mb1"][:, f : f + 1], scale=1.0
        )
        for e in range(EH):
            nc.tensor.matmul(
                h2a[e][:], w2t[:, f, e * P : (e + 1) * P], h1[f][:],
                start=(f == 0), stop=(f == KF - 1),
            )
    for e in range(EH):
        of = p_out.tile([P, Tq], F32, name="outf")
        nc.vector.scalar_tensor_tensor(
            of[:], h2a[e][:], gbt["mb2"][:, e : e + 1], xres[e][:], OP.add, OP.add
        )
        nc.sync.dma_start(d["out_t"][e * P : (e + 1) * P, :], of[:])
    pp.close("h2a_ps", "h1_ps")

    if EH < KE:
        ps_h2b = pp.open("h2b_ps", 1, "PSUM")
        h2b = [ps_h2b.tile([P, Tq], F32, name=f"h2b{e}") for e in range(KE - EH)]
        for f in range(KF):
            for i, e in enumerate(range(EH, KE)):
                nc.tensor.matmul(
                    h2b[i][:], w2t[:, f, e * P : (e + 1) * P], h1[f][:],
                    start=(f == 0), stop=(f == KF - 1),
                )
        for i, e in enumerate(range(EH, KE)):
            of = p_out.tile([P, Tq], F32, name="outf")
            nc.vector.scalar_tensor_tensor(
                of[:], h2b[i][:], gbt["mb2"][:, e : e + 1], xres[e][:], OP.add, OP.add
            )
            nc.sync.dma_start(d["out_t"][e * P : (e + 1) * P, :], of[:])

    pp.close_all()


def build_program(c: Cfg = CFG):
    c.check()
    nc = bacc.Bacc(
        "TRN2",
        target_bir_lowering=False,
        debug=False,
        enable_asserts=False,
        num_devices=c.NC,
    )
    d = {}
    d["xt"] = nc.dram_tensor("xt", [c.E, c.T], BF16, kind="ExternalInput").ap()
    d["x_own"] = nc.dram_tensor("x_own", [c.E, c.Tq], F32, kind="ExternalInput").ap()
    d["wq"] = nc.dram_tensor("wq", [P, c.KE, 2 * P], BF16, kind="ExternalInput").ap()
    d["wk"] = nc.dram_tensor("wk", [P, c.KE, 2 * P], BF16, kind="ExternalInput").ap()
    d["wv"] = nc.dram_tensor("wv", [P, c.KE, 2 * P], BF16, kind="ExternalInput").ap()
    d["wo"] = nc.dram_tensor("wo", [P, c.JC, c.E], BF16, kind="ExternalInput").ap()
    d["w1"] = nc.dram_tensor("w1", [P, c.KE, c.F], BF16, kind="ExternalInput").ap()
    d["w2"] = nc.dram_tensor("w2", [P, c.KF, c.E], BF16, kind="ExternalInput").ap()
    for nm, cols in [
        ("ln1g", c.KE), ("ln1b", c.KE), ("ln2g", c.KE), ("ln2b", c.KE),
        ("boc", c.KE), ("mb1", c.KF), ("mb2", c.KE),
    ]:
        d[nm] = nc.dram_tensor(nm, [P, cols], F32, kind="ExternalInput").ap()
    d["mask01"] = nc.dram_tensor(
        "mask01", [P, 4 * c.Tq], BF16, kind="ExternalInput"
    ).ap()
    d["ident"] = nc.dram_tensor("ident", [P, P], BF16, kind="ExternalInput").ap()
    d["sel64"] = nc.dram_tensor("sel64", [4, 4 * 64], BF16, kind="ExternalInput").ap()
    d["out_t"] = nc.dram_tensor("out_t", [c.E, c.Tq], F32, kind="ExternalOutput").ap()

    with tile.TileContext(nc) as tc:
        _emit(tc, c, d)
    nc.compile()
    return nc


# --------------------------------------------------------------------------
# host side
# --------------------------------------------------------------------------
def _pack_lhsT(w, cols_per_block):
    """[R, C] -> [128, R//128, C] lhsT layout (contiguous DMA)."""
    R, C = w.shape
    assert R % P == 0 and C == cols_per_block
    return np.ascontiguousarray(
        w.reshape(R // P, P, C).transpose(1, 0, 2)
    )


def shard_inputs(inputs, c: Cfg = CFG):
    x = np.ascontiguousarray(np.asarray(inputs["x"], np.float32))
    bf = lambda a: np.ascontiguousarray(np.asarray(a, np.float32)).astype(NPBF16)

    chunks = lambda v, k: np.ascontiguousarray(
        np.asarray(v, np.float32).reshape(k, P).T
    )
    com = {
        "w1": _pack_lhsT(bf(inputs["W1"]), c.F),
        "w2": _pack_lhsT(bf(inputs["W2"]), c.E),
        "ln1g": chunks(inputs["ln1_g"], c.KE),
        "ln1b": chunks(inputs["ln1_b"], c.KE),
        "ln2g": chunks(inputs["ln2_g"], c.KE),
        "ln2b": chunks(inputs["ln2_b"], c.KE),
        "boc": chunks(inputs["bo"], c.KE),
        "mb1": chunks(inputs["b1"], c.KF),
        "mb2": chunks(inputs["b2"], c.KE),
    }

    p_idx = np.arange(P)[:, None]
    tq_idx = np.arange(c.Tq)[None, :]
    mask = np.zeros((P, 4 * c.Tq), np.float32)
    for jb in range(4):
        mask[:, jb * c.Tq : (jb + 1) * c.Tq] = np.where(
            tq_idx >= (jb * P + p_idx), 0.0, -1.0e9
        )
    com["mask01"] = mask.astype(NPBF16)
    com["ident"] = np.eye(P, dtype=np.float32).astype(NPBF16)

    sel = np.zeros((4, 4 * 64), np.float32)
    for h in range(4):
        sel[h, h * 64 : (h + 1) * 64] = 1.0
    com["sel64"] = sel.astype(NPBF16)

    Wq, Wk, Wv = (bf(inputs[k]) for k in ("Wq", "Wk", "Wv"))
    Wo = bf(inputs["Wo"])
    maps = []
    for core in range(c.NC):
        b, r = core // c.CPB, core % c.CPB
        hs = slice(r * 2 * P, (r + 1) * 2 * P)  # this core's 256 head-features
        tok = np.concatenate(
            [np.arange(qc * c.Tq + r * P, qc * c.Tq + r * P + P) for qc in range(c.NQC)]
        )
        m = dict(com)
        m["xt"] = np.ascontiguousarray(x[b].T).astype(NPBF16)
        m["x_own"] = np.ascontiguousarray(x[b][tok, :].T)
        m["wq"] = _pack_lhsT(Wq[:, hs], 2 * P)
        m["wk"] = _pack_lhsT(Wk[:, hs], 2 * P)
        m["wv"] = _pack_lhsT(Wv[:, hs], 2 * P)
        m["wo"] = _pack_lhsT(np.ascontiguousarray(Wo[hs, :]), c.E)
        maps.append(m)
    return maps


def assemble(results, c: Cfg = CFG):
    out = np.empty((c.B, c.T, c.E), np.float32)
    for core in range(c.NC):
        b, r = core // c.CPB, core % c.CPB
        tok = np.concatenate(
            [np.arange(qc * c.Tq + r * P, qc * c.Tq + r * P + P) for qc in range(c.NQC)]
        )
        out[b, tok, :] = results[core]["out_t"].T
    return out


_NC_CACHE = {}


def _get_nc(c: Cfg = CFG):
    if c not in _NC_CACHE:
        _NC_CACHE[c] = build_program(c)
    return _NC_CACHE[c]


LAST_RESULT = None


def kernel(**inputs):
    global LAST_RESULT
    c = CFG
    nc = _get_nc(c)
    maps = shard_inputs(inputs, c)
    res = bass_utils.run_bass_kernel_spmd(nc, maps, core_ids=list(range(c.NC)))
    LAST_RESULT = res
    return assemble(res.results, c)
